# revision 29
# baseline (speedup 1.0000x reference)
"""Sliding-window causal self-attention with RoPE on 8 Trainium2 NeuronCores.

Problem: B=2, S=2048, D=1024, H=16, HD=64, WINDOW=256, fp32 in/out.
Sharding: 2 (batch) x 4 (head-groups of 4 heads). Each core computes its
head-group's QKV projections, RoPE, windowed attention, and a partial output
projection (y_g @ Wo_g.T); the host sums the 4 partials per batch.

v2: all-bf16 compute (inputs, weights, activations in bf16; PSUM accumulation
fp32). Structure:
  - phase 1 (per S-half, weight-stationary): qT/kT projections + inline RoPE
    (P-matmul rotation + 3 DVE ops), v projection into v_aug [64 v-cols +
    ones col per (key-block, head)].
  - attention, query-tile-outer (qt = 128 queries), interleaved with the
    output projection:
      scores^T [k,q] per head for key-block kb=qt over its 384-query window,
      exp on Act (scale=1/8), multiplicative 0/1 window mask post-exp (DVE),
      AV in [q, hd] orientation (lhsT = attn^T slice, rhs = v_aug) so the
      softmax denominator lands per-partition -> tiny reciprocal + per-
      partition-scale Act evacuation, DMA-xbar transpose of y [q,256] into
      yT [256, q], then the Wo matmuls + store for this 128-row seq tile.
Output partials are written bf16; host sums in fp32.
"""
import sys

for _p in ("/opt/trn_rl_repo", "/root/.axon_site/_ro/trn_rl_repo"):
    if _p not in sys.path:
        sys.path.append(_p)

import numpy as np
import ml_dtypes
import concourse.bacc as bacc
import concourse.mybir as mybir
from concourse.tile import TileContext
from concourse.bass_utils import run_bass_kernel_spmd

F32 = mybir.dt.float32
BF16 = mybir.dt.bfloat16
AF = mybir.ActivationFunctionType

B, S, D = 2, 2048, 1024
H, HD = 16, 64
WINDOW = 256
THETA = 10000.0
SCALING = 1.0

HG = 4                      # head-groups (cores per batch)
HPG = H // HG               # heads per group = 4
GD = HPG * HD               # group out width = 256
NKB = S // 128              # 16 key blocks
SCALE = 1.0 / float(np.sqrt(HD))

_CACHE = {}
DEBUG = False


def _build():
    nc = bacc.Bacc(target_bir_lowering=False, trn_type="TRN2")

    xT = nc.dram_tensor("xT", [D, S], BF16, kind="ExternalInput")
    # weights pre-swizzled on host to [128, kt*X] partition-major layout
    wq = nc.dram_tensor("wq", [128, D // 128 * GD], BF16, kind="ExternalInput")
    wk = nc.dram_tensor("wk", [128, D // 128 * GD], BF16, kind="ExternalInput")
    wv = nc.dram_tensor("wv", [128, D // 128 * GD], BF16, kind="ExternalInput")
    wo = nc.dram_tensor("wo", [128, GD // 128 * D], BF16, kind="ExternalInput")
    cos2 = nc.dram_tensor("cos2", [128, S], BF16, kind="ExternalInput")
    sin2 = nc.dram_tensor("sin2", [128, S], BF16, kind="ExternalInput")
    pt2 = nc.dram_tensor("pt2", [128, 128], BF16, kind="ExternalInput")
    # multiplicative 0/1 masks: cols 0:128 diag block, 128:256 far block
    mask = nc.dram_tensor("mask", [128, 256], BF16, kind="ExternalInput")
    ones64 = nc.dram_tensor("ones64", [128, HPG * NKB], BF16, kind="ExternalInput")
    out = nc.dram_tensor("out", [S, D], BF16, kind="ExternalOutput")
    if DEBUG:
        d_qf = nc.dram_tensor("d_qf", [128, 2 * S], BF16, kind="ExternalOutput")
        d_kf = nc.dram_tensor("d_kf", [128, 2 * S], BF16, kind="ExternalOutput")
        d_v = nc.dram_tensor("d_v", [128, NKB * HPG * 65], BF16, kind="ExternalOutput")
        d_attn = nc.dram_tensor("d_attn", [128, HPG * 6 * 384], BF16,
                                kind="ExternalOutput")
        d_yT = nc.dram_tensor("d_yT", [128, 2 * S], BF16, kind="ExternalOutput")

    with TileContext(nc) as tc:
        with tc.tile_pool(name="const", bufs=1) as cpool, \
             tc.tile_pool(name="persist", bufs=1) as ppool, \
             tc.tile_pool(name="x0", bufs=1) as xpool, \
             tc.tile_pool(name="raw", bufs=3) as rawpool, \
             tc.tile_pool(name="tmp", bufs=4) as tpool, \
             tc.tile_pool(name="yq", bufs=3) as ypool, \
             tc.tile_pool(name="ot", bufs=4) as opool, \
             tc.tile_pool(name="rc", bufs=4) as rcpool, \
             tc.tile_pool(name="psA", bufs=1, space="PSUM") as psA, \
             tc.tile_pool(name="psR", bufs=1, space="PSUM") as psR, \
             tc.tile_pool(name="psB", bufs=2, space="PSUM") as psB, \
             tc.tile_pool(name="psC", bufs=1, space="PSUM") as psC:
            # resident weights/constants
            wq_sb = cpool.tile([128, D // 128, GD], BF16)
            wk_sb = cpool.tile([128, D // 128, GD], BF16)
            wv_sb = cpool.tile([128, D // 128, GD], BF16)
            wo_sb = cpool.tile([128, GD // 128, D], BF16)
            cos_sb = cpool.tile([128, S], BF16)
            sin_sb = cpool.tile([128, S], BF16)
            pt2_sb = cpool.tile([128, 128], BF16)
            mask_sb = cpool.tile([128, 256], BF16)
            # per-kt weight chunks interleaved with x rows so the first
            # projection matmuls start ~1.5us in (PE clock ramps early and
            # stays at K=8/8 instead of half-clock through the whole load)
            for kt in range(D // 128):
                nc.scalar.dma_start(wq_sb[:, kt, :], wq.ap()[:, kt * GD:(kt + 1) * GD])
                nc.scalar.dma_start(wk_sb[:, kt, :], wk.ap()[:, kt * GD:(kt + 1) * GD])
            nc.scalar.dma_start(pt2_sb[:], pt2[:])
            nc.scalar.dma_start(cos_sb[:], cos2[:])
            nc.scalar.dma_start(sin_sb[:], sin2[:])
            nc.scalar.dma_start(mask_sb[:], mask[:])

            # persistent activations
            v_sb = ppool.tile([128, NKB * HPG * 65], BF16)
            qf = [ppool.tile([128, S], BF16, name=f"qf{t}") for t in range(2)]
            kf = [ppool.tile([128, S], BF16, name=f"kf{t}") for t in range(2)]
            # attn ring: 6 slots per head, [128 keys, 384 queries] each
            NSLOT = 6
            attn_sb = ppool.tile([128, HPG * NSLOT * 384], BF16)
            yT_sb = ppool.tile([128, 2, S], BF16)

            HS = 1024
            xrow = [[None] * (D // 128) for _ in range(2)]

            def load_x_half(half):
                for kt in range(D // 128):
                    xrow[half][kt] = xpool.tile(
                        [128, HS], BF16, tag=f"x{half}_{kt}",
                        name=f"xrow{half}_{kt}")
                    nc.sync.dma_start(
                        xrow[half][kt][:],
                        xT.ap()[kt * 128:(kt + 1) * 128, half * HS:(half + 1) * HS])

            wsel = [(wq_sb, 0, qf[0]), (wq_sb, 128, qf[1]),
                    (wk_sb, 0, kf[0]), (wk_sb, 128, kf[1])]

            def qk_pass(half, sl):
                """kt-stationary qk projections for one 512-col s-chunk: all
                4 targets accumulate in banks a0-a3 as xrow DMAs land."""
                accs = [psA.tile([128, 512], F32, name=f"acc{half}_{sl}_{u}",
                                 tag=f"a{u}") for u in range(4)]
                for kt in range(D // 128):
                    st, sp = (kt == 0), (kt == D // 128 - 1)
                    for u in range(4):
                        w_t, off, dst = wsel[u]
                        nc.tensor.matmul(
                            accs[u][:], w_t[:, kt, off:off + 128],
                            xrow[half][kt][:, sl * 512:(sl + 1) * 512],
                            start=st, stop=sp)
                return accs

            def rope_block(half, sl, accs, cover):
                """Evacuate + RoPE the 4 qk targets of one s-chunk. After
                each target's chunk, emit one independent cover piece of PE
                work so the single-rot-bank serialization never idles PE."""
                s0 = half * HS + sl * 512
                for u in range(4):
                    w_t, off, dst = wsel[u]
                    raw = rawpool.tile([128, 512], BF16, tag="raw")
                    nc.vector.tensor_copy(raw[:], accs[u][:])
                    rot = psR.tile([128, 512], F32, tag="rot",
                                   name=f"rot{half}_{sl}_{u}")
                    nc.tensor.matmul(rot[:], pt2_sb[:], raw[:],
                                     start=True, stop=True)
                    t1 = tpool.tile([128, 512], BF16, tag="t1")
                    nc.vector.tensor_mul(t1[:], rot[:], sin_sb[:, s0:s0 + 512])
                    t2 = tpool.tile([128, 512], BF16, tag="t2")
                    nc.vector.tensor_mul(t2[:], raw[:], cos_sb[:, s0:s0 + 512])
                    nc.vector.tensor_add(dst[:, s0:s0 + 512], t1[:], t2[:])
                    if cover:
                        cover.pop(0)()
                while cover:
                    cover.pop(0)()

            def qk_target_cover(half, sl, accs_out):
                """Cover pieces for rope: the next s-chunk's projections,
                one piece per target u (uses bank a{u}, which the rope chunk
                just evacuated). Fills accs_out[u] as pieces run."""
                def piece(u):
                    def run():
                        w_t, off, dst = wsel[u]
                        acc = psA.tile([128, 512], F32,
                                       name=f"acc{half}_{sl}_{u}", tag=f"a{u}")
                        for kt in range(D // 128):
                            nc.tensor.matmul(
                                acc[:], w_t[:, kt, off:off + 128],
                                xrow[half][kt][:, sl * 512:(sl + 1) * 512],
                                start=(kt == 0), stop=(kt == D // 128 - 1))
                        accs_out[u] = acc
                    return run
                return [piece(u) for u in range(4)]

            def v_g(half, g):
                """v projection for 4 key blocks (s-subtiles g*4..g*4+3)."""
                vacc = [psA.tile([128, 512], F32, name=f"vacc{half}_{g}_{j}",
                                 tag=f"a{j}") for j in range(2)]
                for kt in range(D // 128):
                    st, sp = (kt == 0), (kt == D // 128 - 1)
                    for j in range(2):
                        for jj in range(2):
                            stl = g * 4 + 2 * j + jj
                            nc.tensor.matmul(
                                vacc[j][:, jj * 256:(jj + 1) * 256],
                                xrow[half][kt][:, stl * 128:(stl + 1) * 128],
                                wv_sb[:, kt, 0:256],
                                start=(st and jj == 0), stop=sp)
                for j in range(2):
                    for jj in range(2):
                        kb = half * 8 + g * 4 + 2 * j + jj
                        dstv = v_sb[:, kb * HPG * 65:(kb + 1) * HPG * 65]
                        nc.scalar.copy(
                            dstv.rearrange("p (g c) -> p g c", c=65)[:, :, 0:64],
                            vacc[j][:, jj * 256:(jj + 1) * 256]
                            .rearrange("p (g c) -> p g c", c=64))

            def v_g_cover(half, g):
                """v_g split into 2 cover pieces (bank a0 then a1) + evacs."""
                vacc = [None, None]

                def mk_mms(j):
                    def run():
                        vacc[j] = psA.tile([128, 512], F32,
                                           name=f"vacc{half}_{g}_{j}",
                                           tag=f"a{j}")
                        for kt in range(D // 128):
                            st, sp = (kt == 0), (kt == D // 128 - 1)
                            for jj in range(2):
                                stl = g * 4 + 2 * j + jj
                                nc.tensor.matmul(
                                    vacc[j][:, jj * 256:(jj + 1) * 256],
                                    xrow[half][kt][:, stl * 128:(stl + 1) * 128],
                                    wv_sb[:, kt, 0:256],
                                    start=(st and jj == 0), stop=sp)
                    return run

                def mk_evac(j):
                    def run():
                        for jj in range(2):
                            kb = half * 8 + g * 4 + 2 * j + jj
                            dstv = v_sb[:, kb * HPG * 65:(kb + 1) * HPG * 65]
                            nc.scalar.copy(
                                dstv.rearrange("p (g c) -> p g c", c=65)[:, :, 0:64],
                                vacc[j][:, jj * 256:(jj + 1) * 256]
                                .rearrange("p (g c) -> p g c", c=64))
                    return run

                return [mk_mms(0), mk_mms(1), mk_evac(0), mk_evac(1)]

            def slot_ap(h, kb):
                base = (h * NSLOT + kb % NSLOT) * 384
                return attn_sb[:, base:base + 384]

            def scores_block(qt):
                """scores + exp + mask for key-block kb=qt."""
                q0 = qt * 128
                n = min(384, S - q0)
                for th in range(2):
                    for i in range(2):
                        h = th * 2 + i
                        ph = 64 * i
                        sc = psB.tile([128, 512], F32, tag="sc",
                                      name=f"sc{qt}_{h}")
                        nc.tensor.matmul(sc[:, 0:n],
                                         kf[th][ph:ph + 64, q0:q0 + 128],
                                         qf[th][ph:ph + 64, q0:q0 + n],
                                         start=True, stop=True)
                        slot = slot_ap(h, qt)
                        nc.scalar.activation(slot[0:128, 0:n], sc[:, 0:n],
                                             AF.Exp, scale=SCALE)
                        eng = nc.vector if h == 0 else nc.gpsimd
                        if n == 384:
                            av = slot.rearrange("p (g c) -> p g c", g=3)[:, 0::2, :]
                            mv = mask_sb[:].rearrange("p (g c) -> p g c", g=2)
                            eng.tensor_mul(av, av, mv)
                        else:
                            eng.tensor_mul(slot[0:128, 0:128],
                                           slot[0:128, 0:128],
                                           mask_sb[:, 0:128])

            def av_block(qt):
                """AV in [q, hd] orientation; den in col 64 of each 65-group."""
                q0 = qt * 128
                acc = psC.tile([128, HPG * 65], F32, tag="accav",
                               name=f"av{qt}")
                parts = [(qt - 2, 256), (qt - 1, 128), (qt, 0)]
                parts = [(kb, ao) for kb, ao in parts if kb >= 0]
                nmm = len(parts) * HPG
                ii = 0
                for h in range(HPG):
                    for kb, ao in parts:
                        sl = slot_ap(h, kb)
                        nc.tensor.matmul(
                            acc[:, h * 65:h * 65 + 65],
                            sl[:, ao:ao + 128],
                            v_sb[:, (kb * HPG + h) * 65:(kb * HPG + h) * 65 + 65],
                            start=(ii == 0), stop=(ii == nmm - 1))
                        ii += 1
                rc = rcpool.tile([128, 4], F32, tag="rc", name=f"rc{qt}")
                den = acc[:].rearrange("p (h c) -> p h c", c=65)[:, :, 64]
                nc.vector.reciprocal_approx_fast(out=rc[:], in_=den)
                yq = ypool.tile([128, 256], BF16, tag="yq", name=f"yq{qt}")
                for h in range(HPG):
                    if h % 2 == 0:
                        nc.scalar.activation(yq[:, h * 64:h * 64 + 64],
                                             acc[:, h * 65:h * 65 + 64],
                                             AF.Copy, scale=rc[:, h:h + 1])
                    else:
                        nc.vector.tensor_scalar_mul(yq[:, h * 64:h * 64 + 64],
                                                    acc[:, h * 65:h * 65 + 64],
                                                    rc[:, h:h + 1])
                # y [q, 256ch] -> yT [256ch, q]: col c lands at [c%128, c//128]
                nc.sync.dma_start_transpose(yT_sb[:, :, q0:q0 + 128], yq[:])

            def out_block(qt):
                """output projection for one 128-row seq tile."""
                q0 = qt * 128
                for dc in range(2):
                    oacc = psA.tile([128, 512], F32, tag=f"a{dc + 2}",
                                    name=f"oacc{qt}_{dc}")
                    for ct in range(2):
                        nc.tensor.matmul(oacc[:],
                                         yT_sb[:, ct, q0:q0 + 128],
                                         wo_sb[:, ct, dc * 512:(dc + 1) * 512],
                                         start=(ct == 0), stop=(ct == 1))
                    ot = opool.tile([128, 512], BF16, tag="ot",
                                    name=f"ot{qt}_{dc}")
                    if dc == 0:
                        nc.scalar.copy(ot[:], oacc[:])
                    else:
                        nc.vector.tensor_copy(ot[:], oacc[:])
                    nc.sync.dma_start(
                        out.ap()[q0:q0 + 128, dc * 512:(dc + 1) * 512], ot[:])

            # ---------------- schedule ----------------
            # One dense PE stream: projections for later s-chunks ride as
            # "cover" pieces inside rope blocks and between qt iterations, so
            # the PE never idles >1us (HAM throttles to half clock after a
            # ~3.4us idle window and the penalty persists).
            # qt pipeline: scores run 2 key-blocks ahead of AV; av/out issue
            # BEFORE scores each iteration so in-order engine queues don't
            # head-of-line block recip/yevac behind masks waiting on exp.
            load_x_half(0)
            nc.scalar.dma_start(wv_sb[:].rearrange("p a b -> p (a b)"), wv.ap())
            nc.gpsimd.memset(
                v_sb[:].rearrange("p (g c) -> p g c", c=65)[:, :, 64], 1.0)
            nc.scalar.dma_start(wo_sb[:].rearrange("p a b -> p (a b)"), wo.ap())

            def qt_iter(qt):
                if 0 <= qt - 2 < NKB:
                    av_block(qt - 2)
                if 0 <= qt - 3 < NKB:
                    out_block(qt - 3)
                if qt < NKB:
                    scores_block(qt)

            acc00 = qk_pass(0, 0)                 # DMA-paced
            load_x_half(1)
            acc01 = [None] * 4
            rope_block(0, 0, acc00, qk_target_cover(0, 1, acc01))
            rope_block(0, 1, acc01, v_g_cover(0, 0))
            qt_iter(0)
            qt_iter(1)
            qt_iter(2)
            qt_iter(3)
            v_g(0, 1)
            qt_iter(4)
            qt_iter(5)
            acc10 = qk_pass(1, 0)
            acc11 = [None] * 4
            rope_block(1, 0, acc10, qk_target_cover(1, 1, acc11))
            rope_block(1, 1, acc11, v_g_cover(1, 0))
            qt_iter(6)
            qt_iter(7)
            qt_iter(8)
            qt_iter(9)
            qt_iter(10)
            qt_iter(11)
            v_g(1, 1)
            for qt in range(12, NKB):
                qt_iter(qt)
            # squeezed tail: av/out close the gap right after the last scores
            av_block(NKB - 2)
            out_block(NKB - 3)
            av_block(NKB - 1)
            out_block(NKB - 2)
            out_block(NKB - 1)

            if DEBUG:
                for t in range(2):
                    nc.sync.dma_start(d_qf[:, t * S:(t + 1) * S], qf[t][:])
                    nc.sync.dma_start(d_kf[:, t * S:(t + 1) * S], kf[t][:])
                nc.sync.dma_start(d_v[:], v_sb[:])
                nc.sync.dma_start(d_attn[:], attn_sb[:])
                nc.sync.dma_start(
                    d_yT[:], yT_sb[:].rearrange("p a b -> p (a b)"))

    nc.finalize()
    return nc


def _rope_tables():
    inv_freq = 1.0 / (THETA ** (np.arange(0, HD, 2, dtype=np.float64) / HD))
    t = np.arange(S, dtype=np.float64) / max(SCALING, 1e-6)
    freqs = np.outer(t, inv_freq)                      # [S, HD/2]
    emb = np.concatenate((freqs, freqs), axis=-1)      # [S, HD]
    return np.cos(emb).astype(np.float32), np.sin(emb).astype(np.float32)


def _swz(w):
    # [kt*128, X] -> [128, kt*X] partition-major contiguous
    kt = w.shape[0] // 128
    return np.ascontiguousarray(
        w.reshape(kt, 128, w.shape[1]).transpose(1, 0, 2).reshape(128, -1))


def _bf16(a):
    return np.ascontiguousarray(a).astype(ml_dtypes.bfloat16)


def _host_prep(x, Wq, Wk, Wv, Wo):
    cos, sin = _rope_tables()
    cosT2 = _bf16(np.tile(cos.T, (2, 1)))     # [128, S]
    sinT2 = _bf16(np.tile(sin.T, (2, 1)))
    P = np.zeros((HD, HD), dtype=np.float32)
    for i in range(HD // 2):
        P[2 * i, 2 * i + 1] = -1.0
        P[2 * i + 1, 2 * i] = 1.0
    PT = P.T
    pt2 = np.zeros((128, 128), dtype=np.float32)
    pt2[0:64, 0:64] = PT
    pt2[64:128, 64:128] = PT

    # multiplicative 0/1 masks in [key 128, query-offset] layout
    kk = np.arange(128)[:, None]
    jj = np.arange(128)[None, :]
    m = np.zeros((128, 256), dtype=np.float32)
    m[:, 0:128] = (jj >= kk)          # diag block: q >= k
    m[:, 128:256] = (jj < kk)         # far block (q = 256+j): q-k < 256
    ones64 = np.ones((128, HPG * NKB), dtype=np.float32)

    in_maps = []
    for c in range(8):
        b, g = c // HG, c % HG
        gsl = slice(g * GD, (g + 1) * GD)
        in_maps.append({
            "xT": _bf16(x[b].T),
            "wq": _bf16(_swz(Wq[gsl, :].T)),
            "wk": _bf16(_swz(Wk[gsl, :].T)),
            "wv": _bf16(_swz(Wv[gsl, :].T)),
            "wo": _bf16(_swz(Wo[:, gsl].T)),
            "cos2": cosT2, "sin2": sinT2, "pt2": _bf16(pt2),
            "mask": _bf16(m), "ones64": _bf16(ones64),
        })
    return in_maps


def _run(inputs, trace=False, **kw):
    if "nc" not in _CACHE:
        _CACHE["nc"] = _build()
    in_maps = _host_prep(inputs["x"], inputs["Wq"], inputs["Wk"],
                         inputs["Wv"], inputs["Wo"])
    return run_bass_kernel_spmd(_CACHE["nc"], in_maps, list(range(8)),
                                trace=trace, **kw)


def kernel(x, Wq, Wk, Wv, Wo):
    res = _run({"x": x, "Wq": Wq, "Wk": Wk, "Wv": Wv, "Wo": Wo})
    out = np.zeros((B, S, D), dtype=np.float32)
    for c in range(8):
        out[c // HG] += np.asarray(res.results[c]["out"]).astype(np.float32)
    return out
